# revision 22
# baseline (speedup 1.0000x reference)
"""Chamfer distance (squared-L2, mean of both directional min-means) on 8
Trainium2 NeuronCores — symmetric single-matmul variant.

Sharding: B=16 batches of N=M=4096 3-D points, data-parallel, 2 batches per
core.  Unlike the two-pass baseline (which ran one matmul per direction,
2x 4096x4096 per batch), each batch's distance matrix is computed ONCE:
  * dist1 (min over columns for each row)   = row-max of -D
  * dist2 (min over rows for each column)   = col-max of -D
so TensorE work and the number of PSUM elements the reducer pipeline must
touch are both halved.  The DVE (the only engine that can do tensor-tensor
min/max; GpSimd elementwise is rejected by walrus on NeuronCore V3) is the
bottleneck at ~2 touches/element: one col-max accumulate + ~1 amortized
row-max tree touch, both in its fp16 2x mode.

Device kernel, per batch (default "tree2" variant):
  * One K=32 stacked bf16 matmul per 128-row chunk emits negated distance
    tiles -D[n, m] into fp32 PSUM ([128, 1024] stripes, 4-deep), with the
    baseline's hi/mid/lo bf16 coordinate splitting: fp32-accurate distances.
  * ScalarE cast-copies each stripe into an fp16 [128, 2, 4096] SBUF tile
    shared by a PAIR of chunks (ScalarE is the only PSUM->SBUF mover; it
    runs ~75% busy, never binding).
  * VectorE, per pair of chunks (fp16 2x tensor_tensor throughout):
      - col-max accumulate acc = max(acc, q) per chunk (first pair
        initializes acc = max(q0, q1) directly, no memset dependency);
      - row-max tree with each level one op spanning BOTH chunks:
        [128,2,2048] <- [128,2,1024] <- [128,2,512] into a batching tile,
        one tensor_reduce per RB=8 chunks -> rm columns.
  * After the 32 chunks: acc [128, 4096] holds per-partition column maxima.
    TensorE transposes it 128x128-tile-wise (identity matmul) into PSUM and
    the DVE reduces the transposed stripes to per-column maxima cm.

The host negates rm/cm (restoring +dist mins), clamps at zero (identical to
the reference's maximum(d, 0): clamping commutes with min) and averages in
f64.

Measured (slope method, R_hi=1200, interleaved rounds, min): ~315 us/pass
vs ~575 us for the two-pass baseline; rel err vs fp32 jax reference ~6e-6.
"""

import os
import sys
from contextlib import ExitStack

import numpy as np

sys.path.insert(0, "/opt/trn_rl_repo")

import ml_dtypes

import concourse.bass as bass
import concourse.tile as tile
from concourse import bacc, mybir
from concourse.bass_utils import run_bass_kernel_spmd

B, N, M = 16, 4096, 4096
NCORES = 8
BPC = B // NCORES          # batches per core
K = 32                     # stacked contraction rows
NCHUNK = N // 128          # 32 output-row chunks per batch
HALF = 2048                # half-stripe width (4 PSUM banks)
BF16 = ml_dtypes.bfloat16
NEG_BIG = -60000.0         # fp16-safe "-inf" for max-accumulators
# Row-max reduction variant (see bench.py):
#   tree2: chunk PAIRS share one cast tile; each tree level is one
#          tensor_tensor spanning both chunks (fewest DVE instructions)
#   tree : colacc + 3-level tensor_tensor halving + batched reduce
#   pair : colacc + 1 halving + reduce(2048)
#   flat : colacc + single reduce(4096)
#   pp   : like tree but col-max accumulator ping-pongs (not in place)
#   notree/nocol: timing probes (partial outputs invalid)
VARIANT = os.environ.get("KM_VARIANT", "prow")
# GpSimd col-max offload (DEAD: walrus rejects TensorTensor/TensorScalarPtr
# on the Pool engine for NeuronCore V3 — kept for reference).  0 = off.
GPD = int(os.environ.get("KM_GPD", "0"))
# Batch the per-chunk final row-max reduces: one tensor_reduce per RB chunks.
RB = int(os.environ.get("KM_RB", "8"))
QBUFS = int(os.environ.get("KM_QBUFS", "6"))
TBUFS = int(os.environ.get("KM_TBUFS", "2"))
# PSUM stripe width: 2048 (2 stripes x 2 bufs) or 1024 (4 stripes x 4 bufs)
PSW = int(os.environ.get("KM_PSW", "1024"))
# prow variant: PSUM stripe width (separate knob)
PPSW = int(os.environ.get("KM_PPSW", "2048"))
# tree4: chunks per tree group (2 or 4)
GRP = int(os.environ.get("KM_GRP", "2"))


# ----------------------------------------------------------------- host prep

def _splitn(x, n):
    """x (fp32/fp64) -> n bf16 arrays p_i with sum(p_i) = x + O(2^-(8n) x)."""
    parts = []
    r = x
    for _ in range(n):
        p = r.astype(BF16)
        parts.append(p)
        r = r - p.astype(x.dtype)
    return parts


def _stacks(z):
    """z: [N, 3] fp32 points -> (lhsT_stack [K, N] bf16, rhs_stack [K, N] bf16).

    Row pairing (lhsT row k multiplies rhs row k, summed over k): the 3-way
    bf16 split of each coordinate (h/m/l) keeps all cross products except
    l.l (2^-32 relative); |z|^2 enters as a 4-way bf16 split against a
    ones-row on the opposite side.  lhsT is globally negated so PSUM
    accumulates -D.
      k 0-8  : (-2 h1).(h2|m2|l2)    k 9-17 : (-2 m1).(h2|m2|l2)
      k 18-23: (-2 l1).(h2|m2)       k 24-27: sq1 parts . 1
      k 28-31: 1 . sq2 parts
    """
    zt = np.ascontiguousarray(z.T.astype(np.float32))          # [3, N]
    h, m, l = _splitn(zt, 3)
    sq = (z.astype(np.float64) ** 2).sum(axis=-1)              # [N]
    sqp = _splitn(sq, 4)
    npts = z.shape[0]

    lhs = np.empty((K, npts), dtype=BF16)
    h2 = (-2.0 * h.astype(np.float32)).astype(BF16)            # exact (power of 2)
    m2 = (-2.0 * m.astype(np.float32)).astype(BF16)
    l2 = (-2.0 * l.astype(np.float32)).astype(BF16)
    for i, a in enumerate((h2, h2, h2, m2, m2, m2, l2, l2)):
        lhs[3 * i: 3 * i + 3] = a
    for i in range(4):
        lhs[24 + i] = sqp[i]
    lhs[28:32] = np.ones((4, npts), dtype=BF16)

    rhs = np.empty((K, npts), dtype=BF16)
    for i, a in enumerate((h, m, l, h, m, l, h, m)):
        rhs[3 * i: 3 * i + 3] = a
    rhs[24:28] = np.ones((4, npts), dtype=BF16)
    for i in range(4):
        rhs[28 + i] = sqp[i]
    return -lhs, rhs           # negated: PSUM accumulates -D, reduce is max


# -------------------------------------------------------------- device build

def _build_nc(repeat=1):
    """repeat > 1 builds a timing variant: the full compute loop re-executes
    `repeat` times inside one NEFF (same data, idempotent: max-accumulators
    are absorbing) so per-pass hardware time can be extracted from the
    wall-clock slope."""
    nc = bacc.Bacc("TRN2", target_bir_lowering=False, debug=False)
    lhs_d = nc.dram_tensor("lhs", [BPC, K, N], mybir.dt.bfloat16,
                           kind="ExternalInput")
    rhs_d = nc.dram_tensor("rhs", [BPC, K, M], mybir.dt.bfloat16,
                           kind="ExternalInput")
    eye_d = nc.dram_tensor("eye", [128, 128], mybir.dt.float16,
                           kind="ExternalInput")
    res_dt = (mybir.dt.float16 if VARIANT in ("prow", "tree4")
              else mybir.dt.float32)
    rm_d = nc.dram_tensor("rowmax", [128, BPC * NCHUNK], res_dt,
                          kind="ExternalOutput")
    cm_d = nc.dram_tensor("colmax", [128, BPC * (M // 128)], res_dt,
                          kind="ExternalOutput")
    lhs_ap, rhs_ap = lhs_d.ap(), rhs_d.ap()

    with tile.TileContext(nc) as tc, ExitStack() as ctx:
        stacks = ctx.enter_context(tc.tile_pool(name="stacks", bufs=1))
        psw = PPSW if VARIANT in ("prow", "tree4") else PSW
        psum = ctx.enter_context(
            tc.tile_pool(name="psum", bufs=4096 // psw, space="PSUM"))
        qpool = ctx.enter_context(tc.tile_pool(name="qcast", bufs=QBUFS))
        tpool = ctx.enter_context(tc.tile_pool(name="tree", bufs=TBUFS))
        apool = ctx.enter_context(tc.tile_pool(name="accs", bufs=1))
        rpool = ctx.enter_context(tc.tile_pool(name="res", bufs=1))

        lhs_t, rhs_t = [], []
        for b in range(BPC):
            lt = stacks.tile([K, N], mybir.dt.bfloat16, tag=f"lhs{b}")
            nc.sync.dma_start(lt[:], lhs_ap[b])
            rt = stacks.tile([K, M], mybir.dt.bfloat16, tag=f"rhs{b}")
            nc.sync.dma_start(rt[:], rhs_ap[b])
            lhs_t.append(lt)
            rhs_t.append(rt)
        eye_t = stacks.tile([128, 128], mybir.dt.float16, tag="eye")
        nc.sync.dma_start(eye_t[:], eye_d.ap())

        rm = rpool.tile([128, BPC * NCHUNK], res_dt, tag="rm")
        cm = rpool.tile([128, BPC * (M // 128)], res_dt, tag="cm")
        ping_pong = VARIANT == "pp"
        acc_bufs = 2 if ping_pong else 1
        accs = []
        nacc = 1 if VARIANT == "tree4" else BPC
        for b in range(nacc):
            bb = []
            for i in range(acc_bufs):
                acc = apool.tile([128, M], mybir.dt.float16,
                                 tag=f"acc{b}_{i}", name=f"acc{b}_{i}")
                nc.gpsimd.memset(acc[:], NEG_BIG)
                bb.append(acc)
            accs.append(bb)
        gaccs = []
        if GPD:
            for b in range(BPC):
                gacc = apool.tile([128, M], mybir.dt.float16,
                                  tag=f"gacc{b}", name=f"gacc{b}")
                nc.gpsimd.memset(gacc[:], NEG_BIG)
                gaccs.append(gacc)

        def body4():
            """tree4: like tree2 (chunk pairs, all-TT tree) but one level
            deeper (L4 to 256 cols/chunk), with ALL row-reduce work deferred
            to a single end-of-kernel tensor_reduce over [128, 64, 256]
            (tree2 pays a DVE op-type-switch penalty ~0.4-2k cycles per
            TT<->reduce transition; tree4 has exactly one transition), and
            the two per-batch col-max finalize reduces merged into one
            [128, 32, 128] reduce each, placed back-to-back at the end."""
            t4 = rpool.tile([128, BPC * NCHUNK, 256], mybir.dt.float16,
                            tag="t4big", name="t4big")
            pts = []
            for b in range(BPC):
                lt, rt = lhs_t[b], rhs_t[b]
                acc = accs[0][0]
                for cp in range(NCHUNK // GRP):
                    qp = qpool.tile([128, GRP, N], mybir.dt.float16, tag="q")
                    for cpar in range(GRP):
                        c = GRP * cp + cpar
                        for h in range(N // PPSW):
                            ps = psum.tile([128, PPSW], mybir.dt.float32,
                                           tag="ps")
                            for j in range(PPSW // 512):
                                nc.tensor.matmul(
                                    ps[:, j * 512:(j + 1) * 512],
                                    lt[:, c * 128:(c + 1) * 128],
                                    rt[:, h * PPSW + j * 512:
                                       h * PPSW + (j + 1) * 512])
                            nc.scalar.copy(
                                qp[:, cpar, h * PPSW:(h + 1) * PPSW], ps[:])
                        if cp > 0 or cpar >= 2:
                            nc.vector.tensor_tensor(
                                acc[:], acc[:], qp[:, cpar],
                                mybir.AluOpType.max)
                        elif cpar == 1:
                            nc.vector.tensor_tensor(
                                acc[:], qp[:, 0], qp[:, 1],
                                mybir.AluOpType.max)
                    # in-place halving tree inside qp (col-accs above already
                    # consumed qp in program order on the same engine)
                    nc.vector.tensor_tensor(
                        qp[:, :, 0:2048], qp[:, :, 0:2048], qp[:, :, 2048:N],
                        mybir.AluOpType.max)
                    nc.vector.tensor_tensor(
                        qp[:, :, 0:1024], qp[:, :, 0:1024],
                        qp[:, :, 1024:2048], mybir.AluOpType.max)
                    nc.vector.tensor_tensor(
                        qp[:, :, 0:512], qp[:, :, 0:512], qp[:, :, 512:1024],
                        mybir.AluOpType.max)
                    g = b * NCHUNK + GRP * cp
                    nc.vector.tensor_tensor(
                        t4[:, g:g + GRP], qp[:, :, 0:256], qp[:, :, 256:512],
                        mybir.AluOpType.max)
                # col-max finalize: transposes into PSUM (reusing stripe
                # slots via the shared tag) + reduces, back-to-back
                facc = accs[0][0]
                for s in range(2):
                    pt = psum.tile([128, 16, 128], mybir.dt.float16,
                                   tag="ps")
                    for j in range(16):
                        nc.tensor.transpose(
                            pt[:, j],
                            facc[:, (s * 16 + j) * 128:
                                 (s * 16 + j + 1) * 128],
                            eye_t[:])
                    nc.vector.tensor_reduce(
                        cm[:, (b * 2 + s) * 16:(b * 2 + s + 1) * 16], pt[:],
                        axis=mybir.AxisListType.X, op=mybir.AluOpType.max)
                # row-max: one reduce per batch; batch 0's hides under
                # batch 1's chunk stream, only batch 1's is tail-exposed
                nc.vector.tensor_reduce(
                    rm[:, b * NCHUNK:(b + 1) * NCHUNK],
                    t4[:, b * NCHUNK:(b + 1) * NCHUNK],
                    axis=mybir.AxisListType.X, op=mybir.AluOpType.max)
            del pts

        def bodyP():
            """prow: per chunk, matmuls fill PPSW-wide fp32 PSUM stripes,
            ScalarE casts them into a [128, N] fp16 SBUF tile, DVE runs one
            col-max accumulate (tensor_tensor, 2x) and one single-window
            max-pool (2x) that writes the chunk's row-max straight into the
            fp16 rm column.  Chunk 0 is cast directly into the accumulator
            (initialization for free, pool reads it there)."""
            for b in range(BPC):
                lt, rt = lhs_t[b], rhs_t[b]
                acc = accs[b][0]
                for c in range(NCHUNK):
                    first = c == 0
                    q = acc if first else qpool.tile(
                        [128, N], mybir.dt.float16, tag="q")
                    for h in range(N // PPSW):
                        ps = psum.tile([128, PPSW], mybir.dt.float32,
                                       tag="ps")
                        for j in range(PPSW // 512):
                            nc.tensor.matmul(
                                ps[:, j * 512:(j + 1) * 512],
                                lt[:, c * 128:(c + 1) * 128],
                                rt[:, h * PPSW + j * 512:
                                   h * PPSW + (j + 1) * 512])
                        nc.scalar.copy(q[:, h * PPSW:(h + 1) * PPSW], ps[:])
                    if not first:
                        nc.vector.tensor_tensor(
                            acc[:], acc[:], q[:], mybir.AluOpType.max)
                    g = b * NCHUNK + c
                    nc.vector.pool(
                        rm[:, g:g + 1],
                        q[:].rearrange("p (a w) -> p a w", w=N),
                        mybir.PoolFunctionType.max)
                _finalize(b, acc)

        def body2():
            """tree2: chunks processed in pairs sharing one [128, 2, N] cast
            tile; each tree level is a single tensor_tensor spanning both
            chunks (halved per-op overhead, fewer DVE instructions)."""
            rbp = max(RB // 2, 1)      # reduce batching in pairs
            if VARIANT == "tree2i":
                order = [(b, cp) for cp in range(NCHUNK // 2)
                         for b in range(BPC)]
            else:
                order = [(b, cp) for b in range(BPC)
                         for cp in range(NCHUNK // 2)]
            t3m = {b: [None] for b in range(BPC)}
            for b, cp in order:
                lt, rt = lhs_t[b], rhs_t[b]
                acc = accs[b][0]
                if True:
                    qp = qpool.tile([128, 2, N], mybir.dt.float16, tag="q")
                    for cpar in range(2):
                        c = 2 * cp + cpar
                        for h in range(N // PSW):
                            ps = psum.tile([128, PSW], mybir.dt.float32,
                                           tag="ps")
                            for j in range(PSW // 512):
                                nc.tensor.matmul(
                                    ps[:, j * 512:(j + 1) * 512],
                                    lt[:, c * 128:(c + 1) * 128],
                                    rt[:, h * PSW + j * 512:
                                       h * PSW + (j + 1) * 512])
                            nc.scalar.copy(
                                qp[:, cpar, h * PSW:(h + 1) * PSW], ps[:])
                        if cp > 0:
                            nc.vector.tensor_tensor(
                                acc[:], acc[:], qp[:, cpar],
                                mybir.AluOpType.max)
                    if cp == 0:
                        # first pair initializes the accumulator directly
                        nc.vector.tensor_tensor(
                            acc[:], qp[:, 0], qp[:, 1], mybir.AluOpType.max)
                    t1 = tpool.tile([128, 2, 2048], mybir.dt.float16,
                                    tag="t1")
                    nc.vector.tensor_tensor(
                        t1[:], qp[:, :, 0:HALF], qp[:, :, HALF:N],
                        mybir.AluOpType.max)
                    t2 = tpool.tile([128, 2, 1024], mybir.dt.float16,
                                    tag="t2")
                    nc.vector.tensor_tensor(
                        t2[:], t1[:, :, 0:1024], t1[:, :, 1024:2048],
                        mybir.AluOpType.max)
                    if cp % rbp == 0:
                        t3m[b][0] = tpool.tile([128, 2 * rbp, 512],
                                               mybir.dt.float16,
                                               tag=f"t3_{b}", name="t3")
                    t3 = t3m[b][0]
                    nc.vector.tensor_tensor(
                        t3[:, 2 * (cp % rbp):2 * (cp % rbp) + 2],
                        t2[:, :, 0:512], t2[:, :, 512:1024],
                        mybir.AluOpType.max)
                    if cp % rbp == rbp - 1:
                        g = b * NCHUNK + 2 * (cp - rbp + 1)
                        nc.vector.tensor_reduce(
                            rm[:, g:g + 2 * rbp], t3[:],
                            axis=mybir.AxisListType.X,
                            op=mybir.AluOpType.max)
                if cp == NCHUNK // 2 - 1:
                    _finalize(b, accs[b][0])

        def _finalize(b, facc):
            if GPD:
                nc.vector.tensor_tensor(
                    facc[:], facc[:], gaccs[b][:], mybir.AluOpType.max)
            for s in range(2):
                pt = psum.tile([128, 16, 128], mybir.dt.float16, tag="ps")
                for j in range(16):
                    nc.tensor.transpose(
                        pt[:, j],
                        facc[:, (s * 16 + j) * 128:(s * 16 + j + 1) * 128],
                        eye_t[:])
                col = (b * 2 + s) * 16
                nc.vector.tensor_reduce(
                    cm[:, col:col + 16], pt[:],
                    axis=mybir.AxisListType.X, op=mybir.AluOpType.max)

        t3s = [None]

        def body():
            for b in range(BPC):
                lt, rt = lhs_t[b], rhs_t[b]
                for c in range(NCHUNK):
                    # chunk 0 casts straight into the accumulator: the first
                    # col-max accumulate becomes a plain initialization and
                    # the row-max tree reads the same values from acc.
                    first = c == 0 and VARIANT not in ("pp", "nocol") \
                        and not GPD
                    if first:
                        q = accs[b][0]
                    else:
                        q = qpool.tile([128, N], mybir.dt.float16, tag="q")
                    for h in range(N // PSW):
                        ps = psum.tile([128, PSW], mybir.dt.float32, tag="ps")
                        for j in range(PSW // 512):
                            nc.tensor.matmul(
                                ps[:, j * 512:(j + 1) * 512],
                                lt[:, c * 128:(c + 1) * 128],
                                rt[:, h * PSW + j * 512:
                                   h * PSW + (j + 1) * 512])
                        nc.scalar.copy(q[:, h * PSW:(h + 1) * PSW], ps[:])
                    # col-max accumulate (elementwise, fp16 2x)
                    if VARIANT != "nocol" and not first:
                        if GPD and c % GPD == GPD - 1:
                            gacc = gaccs[b]
                            nc.gpsimd.tensor_tensor(
                                gacc[:], gacc[:], q[:], mybir.AluOpType.max)
                        elif ping_pong:
                            src = accs[b][c % 2]
                            dst = accs[b][(c + 1) % 2]
                            nc.vector.tensor_tensor(
                                dst[:], src[:], q[:], mybir.AluOpType.max)
                        else:
                            acc = accs[b][0]
                            nc.vector.tensor_tensor(
                                acc[:], acc[:], q[:], mybir.AluOpType.max)
                    # row-max reduction
                    g = b * NCHUNK + c
                    if VARIANT == "notree":
                        pass
                    elif VARIANT == "flat":
                        nc.vector.tensor_reduce(
                            rm[:, g:g + 1], q[:],
                            axis=mybir.AxisListType.X, op=mybir.AluOpType.max)
                    elif VARIANT == "pool":
                        # whole row-max in one DVE max-pool op
                        nc.vector.pool(
                            rm[:, g:g + 1],
                            q[:].rearrange("p (a w) -> p a w", w=N),
                            mybir.PoolFunctionType.max)
                    elif VARIANT == "pool2":
                        # halve with tensor_tensor, then one max-pool
                        t1 = tpool.tile([128, 2048], mybir.dt.float16,
                                        tag="t1")
                        nc.vector.tensor_tensor(
                            t1[:], q[:, 0:HALF], q[:, HALF:N],
                            mybir.AluOpType.max)
                        nc.vector.pool(
                            rm[:, g:g + 1],
                            t1[:].rearrange("p (a w) -> p a w", w=HALF),
                            mybir.PoolFunctionType.max)
                    elif VARIANT == "pair":
                        t1 = tpool.tile([128, 2048], mybir.dt.float16,
                                        tag="t1")
                        nc.vector.tensor_tensor(
                            t1[:], q[:, 0:HALF], q[:, HALF:N],
                            mybir.AluOpType.max)
                        nc.vector.tensor_reduce(
                            rm[:, g:g + 1], t1[:],
                            axis=mybir.AxisListType.X, op=mybir.AluOpType.max)
                    else:                  # tree / pp / nocol
                        t1 = tpool.tile([128, 2048], mybir.dt.float16,
                                        tag="t1")
                        nc.vector.tensor_tensor(
                            t1[:], q[:, 0:HALF], q[:, HALF:N],
                            mybir.AluOpType.max)
                        t2 = tpool.tile([128, 1024], mybir.dt.float16,
                                        tag="t2")
                        nc.vector.tensor_tensor(
                            t2[:], t1[:, 0:1024], t1[:, 1024:2048],
                            mybir.AluOpType.max)
                        if c % RB == 0:
                            t3s[0] = tpool.tile([128, RB, 512],
                                                mybir.dt.float16, tag="t3",
                                                name="t3")
                        t3 = t3s[0]
                        nc.vector.tensor_tensor(
                            t3[:, c % RB], t2[:, 0:512], t2[:, 512:1024],
                            mybir.AluOpType.max)
                        if c % RB == RB - 1:
                            nc.vector.tensor_reduce(
                                rm[:, g - RB + 1:g + 1], t3[:],
                                axis=mybir.AxisListType.X,
                                op=mybir.AluOpType.max)
                # finalize col-max: transpose acc tile-wise, reduce over rows
                facc = accs[b][NCHUNK % 2] if ping_pong else accs[b][0]
                if GPD:
                    nc.vector.tensor_tensor(
                        facc[:], facc[:], gaccs[b][:], mybir.AluOpType.max)
                for s in range(2):
                    pt = psum.tile([128, 16, 128], mybir.dt.float16, tag="ps")
                    for j in range(16):
                        nc.tensor.transpose(
                            pt[:, j],
                            facc[:, (s * 16 + j) * 128:(s * 16 + j + 1) * 128],
                            eye_t[:])
                    col = (b * 2 + s) * 16
                    nc.vector.tensor_reduce(
                        cm[:, col:col + 16], pt[:],
                        axis=mybir.AxisListType.X, op=mybir.AluOpType.max)

        if VARIANT == "prow":
            bfn = bodyP
        elif VARIANT == "tree4":
            bfn = body4
        elif VARIANT == "tree2":
            bfn = body2
        else:
            bfn = body
        if repeat > 1:
            with tc.For_i(0, repeat, 1):
                bfn()
        else:
            bfn()
        nc.sync.dma_start(rm_d.ap(), rm[:])
        nc.sync.dma_start(cm_d.ap(), cm[:])
    nc.compile()
    return nc


_CACHE: dict = {}


def _get_nc():
    if "nc" not in _CACHE:
        _CACHE["nc"] = _build_nc()
    return _CACHE["nc"]


# --------------------------------------------------------------------- entry

def make_in_maps(xyz1, xyz2):
    eye = np.eye(128, dtype=np.float16)
    in_maps = []
    for core in range(NCORES):
        lhs = np.empty((BPC, K, N), dtype=BF16)
        rhs = np.empty((BPC, K, M), dtype=BF16)
        for bl in range(BPC):
            b = core * BPC + bl
            ls, _ = _stacks(np.asarray(xyz1[b]))
            _, rs = _stacks(np.asarray(xyz2[b]))
            lhs[bl], rhs[bl] = ls, rs
        in_maps.append({"lhs": lhs, "rhs": rhs, "eye": eye})
    return in_maps


def combine(results):
    total = 0.0
    for r in results:
        rm = -r["rowmax"].astype(np.float64)   # [128, 64] -> dist1 mins
        cm = -r["colmax"].astype(np.float64)   # [128, 64] -> dist2 mins
        total += np.maximum(rm, 0.0).sum() + np.maximum(cm, 0.0).sum()
    return np.float32(total / (B * N))


def kernel(xyz1, xyz2, **_):
    in_maps = make_in_maps(xyz1, xyz2)
    try:
        res = run_bass_kernel_spmd(_get_nc(), in_maps,
                                   core_ids=list(range(NCORES)))
    except Exception:                      # transient axon/PJRT hiccup
        _CACHE.clear()
        res = run_bass_kernel_spmd(_get_nc(), in_maps,
                                   core_ids=list(range(NCORES)))
    return combine(res.results)

